# revision 10
# baseline (speedup 1.0000x reference)
"""Class-routed autoencoder (moe_routing) Trainium2 kernel.

Strategy:
- The reference computes ALL 10 experts densely then gathers by label; we
  ROUTE instead: sort tokens by class on the host, split every class's tokens
  evenly across the 8 cores (class counts padded up to a multiple of 8 with
  dummy zero tokens), so every core runs an IDENTICAL program (SPMD) on
  N_core = sum_e ceil(c_e/8) tokens laid out as 10 contiguous single-class
  segments. Expert layers slice the right weight block per segment at
  compile time; no gather/scatter on device.
- Everything runs feature-major ([features, tokens]): weights are the
  stationary matmul operand as-is (out = W.T @ x_fm), the batch is the
  moving/free dim, and per-feature bias + ReLU + PSUM->SBUF evacuation fuse
  into one scalar-engine activation op (bias is per-partition).
- Matmuls run in bf16 (weights/x converted on host; fp32 PSUM accumulate,
  biases added in fp32). Final layer output is fp32.
- enc1+enc2 and dec1+dec2 are fused per chunk so the big hidden activations
  (2048-dim) never leave SBUF; h2/e1/e2 (bottleneck dims) are SBUF-resident
  full width. Chunks are a balanced split of N_core with sizes <= 512 (one
  PSUM bank per matmul).
- Expert weights stream through a 4-deep ring; their DMAs have no compute
  dependencies so they prefetch during the encoder phase. Decoder weights
  prefetch during the expert phase.
- Host: permute+transpose x, run 8 cores, inverse-permute the output.
"""

import ml_dtypes
import numpy as np

import concourse.bass as bass
import concourse.mybir as mybir
import concourse.tile as tile
from concourse import bacc
from concourse.bass_utils import run_bass_kernel_spmd

N_CORES = 8
N_CLS = 10
D_IN, D_H, D_BOT, D_EXP = 1024, 2048, 512, 1024

F32 = mybir.dt.float32
BF16 = mybir.dt.bfloat16
RELU = mybir.ActivationFunctionType.Relu
IDENT = mybir.ActivationFunctionType.Identity

CHUNK = 512  # max matmul moving-operand (free dim) size: one PSUM bank fp32


def _chunks(n, step=CHUNK):
    """Balanced split of n into ceil(n/step) near-equal pieces (all <= step)."""
    nch = -(-n // step)
    base, rem = divmod(n, nch)
    out = []
    s = 0
    for i in range(nch):
        sz = base + (1 if i < rem else 0)
        out.append((s, sz))
        s += sz
    return out


def _build(n_seg, n_core):
    """Build the SPMD program for per-class-per-core counts n_seg (sum=n_core)."""
    nc = bacc.Bacc()

    xt = nc.dram_tensor("xt", [D_IN, n_core], BF16, kind="ExternalInput")
    w1 = nc.dram_tensor("w1", [D_IN, D_H], BF16, kind="ExternalInput")
    b1 = nc.dram_tensor("b1", [128, D_H // 128], F32, kind="ExternalInput")
    w2 = nc.dram_tensor("w2", [D_H, D_BOT], BF16, kind="ExternalInput")
    b2 = nc.dram_tensor("b2", [128, D_BOT // 128], F32, kind="ExternalInput")
    ew1 = nc.dram_tensor("ew1", [N_CLS, D_BOT, D_EXP], BF16, kind="ExternalInput")
    eb1 = nc.dram_tensor("eb1", [128, N_CLS, D_EXP // 128], F32, kind="ExternalInput")
    ew2 = nc.dram_tensor("ew2", [N_CLS, D_EXP, D_BOT], BF16, kind="ExternalInput")
    eb2 = nc.dram_tensor("eb2", [128, N_CLS, D_BOT // 128], F32, kind="ExternalInput")
    dw1 = nc.dram_tensor("dw1", [D_BOT, D_H], BF16, kind="ExternalInput")
    db1 = nc.dram_tensor("db1", [128, D_H // 128], F32, kind="ExternalInput")
    dw2 = nc.dram_tensor("dw2", [D_H, D_IN], BF16, kind="ExternalInput")
    db2 = nc.dram_tensor("db2", [128, D_IN // 128], F32, kind="ExternalInput")
    out = nc.dram_tensor("out", [D_IN, n_core], F32, kind="ExternalOutput")

    segs = []  # (class e, col start, col len)
    s = 0
    for e in range(N_CLS):
        if n_seg[e] > 0:
            segs.append((e, s, n_seg[e]))
            s += n_seg[e]
    chunks = _chunks(n_core)

    KT1, MT1 = D_IN // 128, D_H // 128     # enc1: 8, 16
    KT2, MT2 = D_H // 128, D_BOT // 128    # enc2: 16, 4
    KE1, ME1 = D_BOT // 128, D_EXP // 128  # exp1: 4, 8
    KE2, ME2 = D_EXP // 128, D_BOT // 128  # exp2: 8, 4
    KD1, MD1 = D_BOT // 128, D_H // 128    # dec1: 4, 16
    KD2, MD2 = D_H // 128, D_IN // 128     # dec2: 16, 8

    with tile.TileContext(nc) as tc:
        p_const = tc.alloc_tile_pool(name="const", bufs=1)
        p_ps = tc.alloc_tile_pool(name="ps", bufs=8, space="PSUM")

        def bias_tile(h, tag, shape):
            t = p_const.tile(shape, F32, tag=tag, name=tag)
            nc.sync.dma_start(out=t, in_=h[:])
            return t

        b1_t = bias_tile(b1, "b1", [128, MT1])
        b2_t = bias_tile(b2, "b2", [128, MT2])
        eb1_t = bias_tile(eb1, "eb1", [128, N_CLS, ME1])
        eb2_t = bias_tile(eb2, "eb2", [128, N_CLS, ME2])
        db1_t = bias_tile(db1, "db1", [128, MD1])
        db2_t = bias_tile(db2, "db2", [128, MD2])

        # bottleneck activations, SBUF-resident at full width
        p_e2 = tc.alloc_tile_pool(name="e2", bufs=1)
        p_h2 = tc.alloc_tile_pool(name="h2", bufs=1)
        e2_t = [p_e2.tile([128, n_core], BF16, tag=f"e2_{m}", name=f"e2_{m}")
                for m in range(D_BOT // 128)]
        h2_t = [p_h2.tile([128, n_core], BF16, tag=f"h2_{m}", name=f"h2_{m}")
                for m in range(D_BOT // 128)]

        # Expert pool is allocated BEFORE the encoder pool: its space never
        # overlaps encoder tiles, so expert-weight DMAs carry no false deps
        # and prefetch during the encoder phase.
        EW_BUFS = 4
        p_exp = tc.alloc_tile_pool(name="exp", bufs=1)
        e1_t = [p_exp.tile([128, n_core], BF16, tag=f"e1_{m}", name=f"e1_{m}")
                for m in range(D_EXP // 128)]

        # ---------------- encoder (fused enc1+enc2 per chunk) -----------------
        p_enc = tc.alloc_tile_pool(name="enc", bufs=1)

        def load_xc(c0, cl):
            xc = []
            for k in range(KT1):
                t = p_enc.tile([128, CHUNK], BF16, tag="xc", name="xc", bufs=KT1)
                nc.sync.dma_start(out=t[:, :cl],
                                  in_=xt[k * 128:(k + 1) * 128, c0:c0 + cl])
                xc.append(t)
            return xc

        # x chunk 0 first so the PE can start quickly, then W1; W2 loads are
        # deferred past chunk-0 enc1 (not needed until enc2).
        xc = load_xc(*chunks[0])
        w1_t = []
        for k in range(KT1):
            t = p_enc.tile([128, D_H], BF16, tag=f"w1_{k}", name=f"w1_{k}")
            nc.sync.dma_start(out=t, in_=w1[k * 128:(k + 1) * 128, :])
            w1_t.append(t)
        w2_t = []

        for ci, (c0, cl) in enumerate(chunks):
            if ci > 0:
                xc = load_xc(c0, cl)
            h1c = []
            for m in range(MT1):
                ps = p_ps.tile([128, cl], F32, tag="ps", name="ps")
                for k in range(KT1):
                    nc.tensor.matmul(ps, w1_t[k][:, m * 128:(m + 1) * 128],
                                     xc[k][:, :cl],
                                     start=(k == 0), stop=(k == KT1 - 1))
                h = p_enc.tile([128, CHUNK], BF16, tag="h1c", name="h1c",
                               bufs=MT1)
                nc.scalar.activation(out=h[:, :cl], in_=ps, func=RELU,
                                     bias=b1_t[:, m:m + 1], scale=1.0)
                h1c.append(h)
            if ci == 0:
                for k in range(KT2):
                    t = p_enc.tile([128, D_BOT], BF16, tag=f"w2_{k}",
                                   name=f"w2_{k}")
                    nc.sync.dma_start(out=t, in_=w2[k * 128:(k + 1) * 128, :])
                    w2_t.append(t)
            for m in range(MT2):
                ps = p_ps.tile([128, cl], F32, tag="ps", name="ps")
                for k in range(KT2):
                    nc.tensor.matmul(ps, w2_t[k][:, m * 128:(m + 1) * 128],
                                     h1c[k][:, :cl],
                                     start=(k == 0), stop=(k == KT2 - 1))
                nc.scalar.activation(out=h2_t[m][:, c0:c0 + cl], in_=ps, func=RELU,
                                     bias=b2_t[:, m:m + 1], scale=1.0)

        p_enc.release()

        # Decoder pool reuses the released encoder space; its weight DMAs only
        # wait on the last encoder consumers, so they stream during the expert
        # phase.
        p_dec = tc.alloc_tile_pool(name="dec", bufs=1)
        # Triggered from the (otherwise idle) gpsimd sequencer: these waits on
        # the encoder's last consumers must not head-of-line-block the expert
        # weight stream on the sync sequencer.
        dw1_t = []
        for k in range(KD1):
            t = p_dec.tile([128, D_H], BF16, tag=f"dw1_{k}", name=f"dw1_{k}")
            nc.gpsimd.dma_start(out=t, in_=dw1[k * 128:(k + 1) * 128, :])
            dw1_t.append(t)
        dw2_t = []
        for k in range(KD2):
            t = p_dec.tile([128, D_IN], BF16, tag=f"dw2_{k}", name=f"dw2_{k}")
            nc.gpsimd.dma_start(out=t, in_=dw2[k * 128:(k + 1) * 128, :])
            dw2_t.append(t)

        # ---------------- experts: h2[512] -> e1[1024] -> e2[512] -------------
        for e, s0, sl in segs:
            ew1_t = p_exp.tile([128, KE1, D_EXP], BF16, tag="ew1", name="ew1",
                               bufs=EW_BUFS)
            nc.sync.dma_start(
                out=ew1_t, in_=ew1[e].rearrange("(a p) n -> p a n", p=128))
            ew2_t = p_exp.tile([128, KE2, D_BOT], BF16, tag="ew2", name="ew2",
                               bufs=EW_BUFS)
            nc.sync.dma_start(
                out=ew2_t, in_=ew2[e].rearrange("(a p) n -> p a n", p=128))

            for c0, cl in _chunks(sl):
                a, al = s0 + c0, cl
                for m in range(ME1):
                    ps = p_ps.tile([128, al], F32, tag="ps", name="ps")
                    for k in range(KE1):
                        nc.tensor.matmul(ps, ew1_t[:, k, m * 128:(m + 1) * 128],
                                         h2_t[k][:, a:a + al],
                                         start=(k == 0), stop=(k == KE1 - 1))
                    nc.scalar.activation(out=e1_t[m][:, a:a + al], in_=ps,
                                         func=RELU, bias=eb1_t[:, e, m:m + 1],
                                         scale=1.0)
                for m in range(ME2):
                    ps = p_ps.tile([128, al], F32, tag="ps", name="ps")
                    for k in range(KE2):
                        nc.tensor.matmul(ps, ew2_t[:, k, m * 128:(m + 1) * 128],
                                         e1_t[k][:, a:a + al],
                                         start=(k == 0), stop=(k == KE2 - 1))
                    nc.scalar.activation(out=e2_t[m][:, a:a + al], in_=ps,
                                         func=RELU, bias=eb2_t[:, e, m:m + 1],
                                         scale=1.0)

        # ---------------- decoder (fused dec1+dec2 per chunk) -----------------
        for c0, cl in chunks:
            d1c = []
            for m in range(MD1):
                ps = p_ps.tile([128, cl], F32, tag="ps", name="ps")
                for k in range(KD1):
                    nc.tensor.matmul(ps, dw1_t[k][:, m * 128:(m + 1) * 128],
                                     e2_t[k][:, c0:c0 + cl],
                                     start=(k == 0), stop=(k == KD1 - 1))
                d = p_dec.tile([128, CHUNK], BF16, tag="d1c", name="d1c",
                               bufs=MD1)
                nc.scalar.activation(out=d[:, :cl], in_=ps, func=RELU,
                                     bias=db1_t[:, m:m + 1], scale=1.0)
                d1c.append(d)
            for m in range(MD2):
                ps = p_ps.tile([128, cl], F32, tag="ps", name="ps")
                for k in range(KD2):
                    nc.tensor.matmul(ps, dw2_t[k][:, m * 128:(m + 1) * 128],
                                     d1c[k][:, :cl],
                                     start=(k == 0), stop=(k == KD2 - 1))
                o_t = p_dec.tile([128, CHUNK], F32, tag="o", name="o", bufs=4)
                nc.scalar.activation(out=o_t[:, :cl], in_=ps, func=IDENT,
                                     bias=db2_t[:, m:m + 1], scale=1.0)
                nc.sync.dma_start(out=out[m * 128:(m + 1) * 128, c0:c0 + cl],
                                  in_=o_t[:, :cl])

        p_dec.release()
        p_exp.release()
        p_h2.release()
        p_e2.release()
        p_ps.release()
        p_const.release()

    nc.finalize()
    return nc


_CACHE = {}


def _get_nc(n_seg, n_core):
    key = tuple(n_seg)
    if key not in _CACHE:
        _CACHE[key] = _build(n_seg, n_core)
    return _CACHE[key]


def _bf16(a):
    return np.ascontiguousarray(np.asarray(a, np.float32).astype(ml_dtypes.bfloat16))


def _f32(a):
    return np.ascontiguousarray(np.asarray(a, np.float32))


def _bias_fm(b, mt):
    """[mt*128] -> [128, mt] feature-major (partition-contiguous) layout."""
    return np.ascontiguousarray(np.asarray(b, np.float32).reshape(mt, 128).T)


def _ebias_fm(b, mt):
    """[N_CLS, mt*128] -> [128, N_CLS, mt]."""
    a = np.asarray(b, np.float32).reshape(N_CLS, mt, 128)
    return np.ascontiguousarray(a.transpose(2, 0, 1))


def kernel(x, labels, W1, b1, W2, b2, EW1, Eb1, EW2, Eb2, DW1, Db1, DW2, Db2):
    x = np.asarray(x, dtype=np.float32)
    labels_np = np.asarray(labels).astype(np.int64)
    B = x.shape[0]

    counts = np.bincount(labels_np, minlength=N_CLS)
    n_seg = [int(-(-int(c) // N_CORES)) for c in counts]  # ceil(c/8)
    n_core = int(sum(n_seg))

    # assign tokens: class e sorted tokens padded to 8*n_seg[e], row j -> core j
    order = np.argsort(labels_np, kind="stable")
    idx_by_class = np.split(order, np.cumsum(counts)[:-1])
    core_tok = np.full((N_CORES, n_core), -1, dtype=np.int64)
    off = 0
    for e in range(N_CLS):
        ne = n_seg[e]
        if ne == 0:
            continue
        padded = np.full(N_CORES * ne, -1, dtype=np.int64)
        padded[:counts[e]] = idx_by_class[e]
        core_tok[:, off:off + ne] = padded.reshape(N_CORES, ne)
        off += ne

    weights = {
        "w1": _bf16(W1), "b1": _bias_fm(b1, D_H // 128),
        "w2": _bf16(W2), "b2": _bias_fm(b2, D_BOT // 128),
        "ew1": _bf16(EW1), "eb1": _ebias_fm(Eb1, D_EXP // 128),
        "ew2": _bf16(EW2), "eb2": _ebias_fm(Eb2, D_BOT // 128),
        "dw1": _bf16(DW1), "db1": _bias_fm(Db1, D_H // 128),
        "dw2": _bf16(DW2), "db2": _bias_fm(Db2, D_IN // 128),
    }

    x_bf = x.astype(ml_dtypes.bfloat16)
    in_maps = []
    for j in range(N_CORES):
        ids = core_tok[j]
        valid = ids >= 0
        xc = np.zeros((n_core, D_IN), dtype=ml_dtypes.bfloat16)
        xc[valid] = x_bf[ids[valid]]
        im = {"xt": np.ascontiguousarray(xc.T)}
        im.update(weights)
        in_maps.append(im)

    nc = _get_nc(n_seg, n_core)
    res = run_bass_kernel_spmd(nc, in_maps, core_ids=list(range(N_CORES)))

    out = np.empty((B, D_IN), dtype=np.float32)
    for j in range(N_CORES):
        oc = res.results[j]["out"]  # [D_IN, n_core]
        ids = core_tok[j]
        valid = ids >= 0
        out[ids[valid]] = oc.T[valid]
    return out


# revision 11
# speedup vs baseline: 1.0061x; 1.0061x over previous
"""Class-routed autoencoder (moe_routing) Trainium2 kernel.

Strategy:
- The reference computes ALL 10 experts densely then gathers by label; we
  ROUTE instead: sort tokens by class on the host, split every class's tokens
  evenly across the 8 cores (class counts padded up to a multiple of 8 with
  dummy zero tokens), so every core runs an IDENTICAL program (SPMD) on
  N_core = sum_e ceil(c_e/8) tokens laid out as 10 contiguous single-class
  segments. Expert layers slice the right weight block per segment at
  compile time; no gather/scatter on device.
- Everything runs feature-major ([features, tokens]): weights are the
  stationary matmul operand as-is (out = W.T @ x_fm), the batch is the
  moving/free dim, and per-feature bias + ReLU + PSUM->SBUF evacuation fuse
  into one scalar-engine activation op (bias is per-partition).
- Matmuls run in bf16 (weights/x converted on host; fp32 PSUM accumulate,
  biases added in fp32). Final layer output is fp32.
- enc1+enc2 and dec1+dec2 are fused per chunk so the big hidden activations
  (2048-dim) never leave SBUF; h2/e1/e2 (bottleneck dims) are SBUF-resident
  full width. Chunks are a balanced split of N_core with sizes <= 512 (one
  PSUM bank per matmul).
- Expert weights stream through a 4-deep ring; their DMAs have no compute
  dependencies so they prefetch during the encoder phase. Decoder weights
  prefetch during the expert phase.
- Host: permute+transpose x, run 8 cores, inverse-permute the output.
"""

import ml_dtypes
import numpy as np

import concourse.bass as bass
import concourse.mybir as mybir
import concourse.tile as tile
from concourse import bacc
from concourse.bass_utils import run_bass_kernel_spmd

N_CORES = 8
N_CLS = 10
D_IN, D_H, D_BOT, D_EXP = 1024, 2048, 512, 1024

F32 = mybir.dt.float32
BF16 = mybir.dt.bfloat16
RELU = mybir.ActivationFunctionType.Relu
IDENT = mybir.ActivationFunctionType.Identity

CHUNK = 512  # max matmul moving-operand (free dim) size: one PSUM bank fp32


def _chunks(n, step=CHUNK):
    """Balanced split of n into ceil(n/step) near-equal pieces (all <= step)."""
    nch = -(-n // step)
    base, rem = divmod(n, nch)
    out = []
    s = 0
    for i in range(nch):
        sz = base + (1 if i < rem else 0)
        out.append((s, sz))
        s += sz
    return out


def _build(n_seg, n_core):
    """Build the SPMD program for per-class-per-core counts n_seg (sum=n_core)."""
    nc = bacc.Bacc()

    xt = nc.dram_tensor("xt", [D_IN, n_core], BF16, kind="ExternalInput")
    w1 = nc.dram_tensor("w1", [D_IN, D_H], BF16, kind="ExternalInput")
    b1 = nc.dram_tensor("b1", [128, D_H // 128], F32, kind="ExternalInput")
    w2 = nc.dram_tensor("w2", [D_H, D_BOT], BF16, kind="ExternalInput")
    b2 = nc.dram_tensor("b2", [128, D_BOT // 128], F32, kind="ExternalInput")
    ew1 = nc.dram_tensor("ew1", [N_CLS, D_BOT, D_EXP], BF16, kind="ExternalInput")
    eb1 = nc.dram_tensor("eb1", [128, N_CLS, D_EXP // 128], F32, kind="ExternalInput")
    ew2 = nc.dram_tensor("ew2", [N_CLS, D_EXP, D_BOT], BF16, kind="ExternalInput")
    eb2 = nc.dram_tensor("eb2", [128, N_CLS, D_BOT // 128], F32, kind="ExternalInput")
    dw1 = nc.dram_tensor("dw1", [D_BOT, D_H], BF16, kind="ExternalInput")
    db1 = nc.dram_tensor("db1", [128, D_H // 128], F32, kind="ExternalInput")
    dw2 = nc.dram_tensor("dw2", [D_H, D_IN], BF16, kind="ExternalInput")
    db2 = nc.dram_tensor("db2", [128, D_IN // 128], F32, kind="ExternalInput")
    out = nc.dram_tensor("out", [D_IN, n_core], F32, kind="ExternalOutput")

    segs = []  # (class e, col start, col len)
    s = 0
    for e in range(N_CLS):
        if n_seg[e] > 0:
            segs.append((e, s, n_seg[e]))
            s += n_seg[e]
    chunks = _chunks(n_core)
    XC_BUFS = (D_IN // 128) * min(len(chunks), 4)

    KT1, MT1 = D_IN // 128, D_H // 128     # enc1: 8, 16
    KT2, MT2 = D_H // 128, D_BOT // 128    # enc2: 16, 4
    KE1, ME1 = D_BOT // 128, D_EXP // 128  # exp1: 4, 8
    KE2, ME2 = D_EXP // 128, D_BOT // 128  # exp2: 8, 4
    KD1, MD1 = D_BOT // 128, D_H // 128    # dec1: 4, 16
    KD2, MD2 = D_H // 128, D_IN // 128     # dec2: 16, 8

    with tile.TileContext(nc) as tc:
        p_const = tc.alloc_tile_pool(name="const", bufs=1)
        p_ps = tc.alloc_tile_pool(name="ps", bufs=8, space="PSUM")

        def bias_tile(h, tag, shape):
            t = p_const.tile(shape, F32, tag=tag, name=tag)
            nc.sync.dma_start(out=t, in_=h[:])
            return t

        b1_t = bias_tile(b1, "b1", [128, MT1])
        b2_t = bias_tile(b2, "b2", [128, MT2])
        eb1_t = bias_tile(eb1, "eb1", [128, N_CLS, ME1])
        eb2_t = bias_tile(eb2, "eb2", [128, N_CLS, ME2])
        db1_t = bias_tile(db1, "db1", [128, MD1])
        db2_t = bias_tile(db2, "db2", [128, MD2])

        # bottleneck activations, SBUF-resident at full width
        p_e2 = tc.alloc_tile_pool(name="e2", bufs=1)
        p_h2 = tc.alloc_tile_pool(name="h2", bufs=1)
        e2_t = [p_e2.tile([128, n_core], BF16, tag=f"e2_{m}", name=f"e2_{m}")
                for m in range(D_BOT // 128)]
        h2_t = [p_h2.tile([128, n_core], BF16, tag=f"h2_{m}", name=f"h2_{m}")
                for m in range(D_BOT // 128)]

        # Expert pool is allocated BEFORE the encoder pool: its space never
        # overlaps encoder tiles, so expert-weight DMAs carry no false deps
        # and prefetch during the encoder phase.
        EW_BUFS = 4
        p_exp = tc.alloc_tile_pool(name="exp", bufs=1)
        e1_t = [p_exp.tile([128, n_core], BF16, tag=f"e1_{m}", name=f"e1_{m}")
                for m in range(D_EXP // 128)]

        # ---------------- encoder (fused enc1+enc2 per chunk) -----------------
        p_enc = tc.alloc_tile_pool(name="enc", bufs=1)

        def load_xc(c0, cl):
            xc = []
            for k in range(KT1):
                t = p_enc.tile([128, CHUNK], BF16, tag="xc", name="xc", bufs=XC_BUFS)
                nc.sync.dma_start(out=t[:, :cl],
                                  in_=xt[k * 128:(k + 1) * 128, c0:c0 + cl])
                xc.append(t)
            return xc

        # x chunk 0 first so the PE can start quickly, then W1; W2 loads are
        # deferred past chunk-0 enc1 (not needed until enc2).
        xc = load_xc(*chunks[0])
        w1_t = []
        for k in range(KT1):
            t = p_enc.tile([128, D_H], BF16, tag=f"w1_{k}", name=f"w1_{k}")
            nc.sync.dma_start(out=t, in_=w1[k * 128:(k + 1) * 128, :])
            w1_t.append(t)
        w2_t = []

        for ci, (c0, cl) in enumerate(chunks):
            if ci > 0:
                xc = load_xc(c0, cl)
            h1c = []
            for m in range(MT1):
                ps = p_ps.tile([128, cl], F32, tag="ps", name="ps")
                for k in range(KT1):
                    nc.tensor.matmul(ps, w1_t[k][:, m * 128:(m + 1) * 128],
                                     xc[k][:, :cl],
                                     start=(k == 0), stop=(k == KT1 - 1))
                h = p_enc.tile([128, CHUNK], BF16, tag="h1c", name="h1c",
                               bufs=MT1)
                nc.scalar.activation(out=h[:, :cl], in_=ps, func=RELU,
                                     bias=b1_t[:, m:m + 1], scale=1.0)
                h1c.append(h)
            if ci == 0:
                for k in range(KT2):
                    t = p_enc.tile([128, D_BOT], BF16, tag=f"w2_{k}",
                                   name=f"w2_{k}")
                    nc.sync.dma_start(out=t, in_=w2[k * 128:(k + 1) * 128, :])
                    w2_t.append(t)
            for m in range(MT2):
                ps = p_ps.tile([128, cl], F32, tag="ps", name="ps")
                for k in range(KT2):
                    nc.tensor.matmul(ps, w2_t[k][:, m * 128:(m + 1) * 128],
                                     h1c[k][:, :cl],
                                     start=(k == 0), stop=(k == KT2 - 1))
                nc.scalar.activation(out=h2_t[m][:, c0:c0 + cl], in_=ps, func=RELU,
                                     bias=b2_t[:, m:m + 1], scale=1.0)

        p_enc.release()

        # Decoder pool reuses the released encoder space; its weight DMAs only
        # wait on the last encoder consumers, so they stream during the expert
        # phase.
        p_dec = tc.alloc_tile_pool(name="dec", bufs=1)
        # Triggered from the (otherwise idle) gpsimd sequencer: these waits on
        # the encoder's last consumers must not head-of-line-block the expert
        # weight stream on the sync sequencer.
        dw1_t = []
        for k in range(KD1):
            t = p_dec.tile([128, D_H], BF16, tag=f"dw1_{k}", name=f"dw1_{k}")
            nc.gpsimd.dma_start(out=t, in_=dw1[k * 128:(k + 1) * 128, :])
            dw1_t.append(t)
        dw2_t = []
        for k in range(KD2):
            t = p_dec.tile([128, D_IN], BF16, tag=f"dw2_{k}", name=f"dw2_{k}")
            nc.gpsimd.dma_start(out=t, in_=dw2[k * 128:(k + 1) * 128, :])
            dw2_t.append(t)

        # ---------------- experts: h2[512] -> e1[1024] -> e2[512] -------------
        for e, s0, sl in segs:
            ew1_t = p_exp.tile([128, KE1, D_EXP], BF16, tag="ew1", name="ew1",
                               bufs=EW_BUFS)
            nc.sync.dma_start(
                out=ew1_t, in_=ew1[e].rearrange("(a p) n -> p a n", p=128))
            ew2_t = p_exp.tile([128, KE2, D_BOT], BF16, tag="ew2", name="ew2",
                               bufs=EW_BUFS)
            nc.sync.dma_start(
                out=ew2_t, in_=ew2[e].rearrange("(a p) n -> p a n", p=128))

            for c0, cl in _chunks(sl):
                a, al = s0 + c0, cl
                for m in range(ME1):
                    ps = p_ps.tile([128, al], F32, tag="ps", name="ps")
                    for k in range(KE1):
                        nc.tensor.matmul(ps, ew1_t[:, k, m * 128:(m + 1) * 128],
                                         h2_t[k][:, a:a + al],
                                         start=(k == 0), stop=(k == KE1 - 1))
                    nc.scalar.activation(out=e1_t[m][:, a:a + al], in_=ps,
                                         func=RELU, bias=eb1_t[:, e, m:m + 1],
                                         scale=1.0)
                for m in range(ME2):
                    ps = p_ps.tile([128, al], F32, tag="ps", name="ps")
                    for k in range(KE2):
                        nc.tensor.matmul(ps, ew2_t[:, k, m * 128:(m + 1) * 128],
                                         e1_t[k][:, a:a + al],
                                         start=(k == 0), stop=(k == KE2 - 1))
                    nc.scalar.activation(out=e2_t[m][:, a:a + al], in_=ps,
                                         func=RELU, bias=eb2_t[:, e, m:m + 1],
                                         scale=1.0)

        # ---------------- decoder (fused dec1+dec2 per chunk) -----------------
        for c0, cl in chunks:
            d1c = []
            for m in range(MD1):
                ps = p_ps.tile([128, cl], F32, tag="ps", name="ps")
                for k in range(KD1):
                    nc.tensor.matmul(ps, dw1_t[k][:, m * 128:(m + 1) * 128],
                                     e2_t[k][:, c0:c0 + cl],
                                     start=(k == 0), stop=(k == KD1 - 1))
                d = p_dec.tile([128, CHUNK], BF16, tag="d1c", name="d1c",
                               bufs=MD1)
                nc.scalar.activation(out=d[:, :cl], in_=ps, func=RELU,
                                     bias=db1_t[:, m:m + 1], scale=1.0)
                d1c.append(d)
            for m in range(MD2):
                ps = p_ps.tile([128, cl], F32, tag="ps", name="ps")
                for k in range(KD2):
                    nc.tensor.matmul(ps, dw2_t[k][:, m * 128:(m + 1) * 128],
                                     d1c[k][:, :cl],
                                     start=(k == 0), stop=(k == KD2 - 1))
                o_t = p_dec.tile([128, CHUNK], F32, tag="o", name="o", bufs=4)
                nc.scalar.activation(out=o_t[:, :cl], in_=ps, func=IDENT,
                                     bias=db2_t[:, m:m + 1], scale=1.0)
                nc.sync.dma_start(out=out[m * 128:(m + 1) * 128, c0:c0 + cl],
                                  in_=o_t[:, :cl])

        p_dec.release()
        p_exp.release()
        p_h2.release()
        p_e2.release()
        p_ps.release()
        p_const.release()

    nc.finalize()
    return nc


_CACHE = {}


def _get_nc(n_seg, n_core):
    key = tuple(n_seg)
    if key not in _CACHE:
        _CACHE[key] = _build(n_seg, n_core)
    return _CACHE[key]


def _bf16(a):
    return np.ascontiguousarray(np.asarray(a, np.float32).astype(ml_dtypes.bfloat16))


def _f32(a):
    return np.ascontiguousarray(np.asarray(a, np.float32))


def _bias_fm(b, mt):
    """[mt*128] -> [128, mt] feature-major (partition-contiguous) layout."""
    return np.ascontiguousarray(np.asarray(b, np.float32).reshape(mt, 128).T)


def _ebias_fm(b, mt):
    """[N_CLS, mt*128] -> [128, N_CLS, mt]."""
    a = np.asarray(b, np.float32).reshape(N_CLS, mt, 128)
    return np.ascontiguousarray(a.transpose(2, 0, 1))


def kernel(x, labels, W1, b1, W2, b2, EW1, Eb1, EW2, Eb2, DW1, Db1, DW2, Db2):
    x = np.asarray(x, dtype=np.float32)
    labels_np = np.asarray(labels).astype(np.int64)
    B = x.shape[0]

    counts = np.bincount(labels_np, minlength=N_CLS)
    n_seg = [int(-(-int(c) // N_CORES)) for c in counts]  # ceil(c/8)
    n_core = int(sum(n_seg))

    # assign tokens: class e sorted tokens padded to 8*n_seg[e], row j -> core j
    order = np.argsort(labels_np, kind="stable")
    idx_by_class = np.split(order, np.cumsum(counts)[:-1])
    core_tok = np.full((N_CORES, n_core), -1, dtype=np.int64)
    off = 0
    for e in range(N_CLS):
        ne = n_seg[e]
        if ne == 0:
            continue
        padded = np.full(N_CORES * ne, -1, dtype=np.int64)
        padded[:counts[e]] = idx_by_class[e]
        core_tok[:, off:off + ne] = padded.reshape(N_CORES, ne)
        off += ne

    weights = {
        "w1": _bf16(W1), "b1": _bias_fm(b1, D_H // 128),
        "w2": _bf16(W2), "b2": _bias_fm(b2, D_BOT // 128),
        "ew1": _bf16(EW1), "eb1": _ebias_fm(Eb1, D_EXP // 128),
        "ew2": _bf16(EW2), "eb2": _ebias_fm(Eb2, D_BOT // 128),
        "dw1": _bf16(DW1), "db1": _bias_fm(Db1, D_H // 128),
        "dw2": _bf16(DW2), "db2": _bias_fm(Db2, D_IN // 128),
    }

    x_bf = x.astype(ml_dtypes.bfloat16)
    in_maps = []
    for j in range(N_CORES):
        ids = core_tok[j]
        valid = ids >= 0
        xc = np.zeros((n_core, D_IN), dtype=ml_dtypes.bfloat16)
        xc[valid] = x_bf[ids[valid]]
        im = {"xt": np.ascontiguousarray(xc.T)}
        im.update(weights)
        in_maps.append(im)

    nc = _get_nc(n_seg, n_core)
    res = run_bass_kernel_spmd(nc, in_maps, core_ids=list(range(N_CORES)))

    out = np.empty((B, D_IN), dtype=np.float32)
    for j in range(N_CORES):
        oc = res.results[j]["out"]  # [D_IN, n_core]
        ids = core_tok[j]
        valid = ids >= 0
        out[ids[valid]] = oc.T[valid]
    return out


# revision 12
# speedup vs baseline: 1.0174x; 1.0113x over previous
"""Class-routed autoencoder (moe_routing) Trainium2 kernel.

Strategy:
- The reference computes ALL 10 experts densely then gathers by label; we
  ROUTE instead: sort tokens by class on the host, split every class's tokens
  evenly across the 8 cores (class counts padded up to a multiple of 8 with
  dummy zero tokens), so every core runs an IDENTICAL program (SPMD) on
  N_core = sum_e ceil(c_e/8) tokens laid out as 10 contiguous single-class
  segments. Expert layers slice the right weight block per segment at
  compile time; no gather/scatter on device.
- Everything runs feature-major ([features, tokens]): weights are the
  stationary matmul operand as-is (out = W.T @ x_fm), the batch is the
  moving/free dim, and per-feature bias + ReLU + PSUM->SBUF evacuation fuse
  into one scalar-engine activation op (bias is per-partition).
- Matmuls run in bf16 (weights/x converted on host; fp32 PSUM accumulate,
  biases added in fp32). Final layer output is fp32.
- enc1+enc2 and dec1+dec2 are fused per chunk so the big hidden activations
  (2048-dim) never leave SBUF; h2/e1/e2 (bottleneck dims) are SBUF-resident
  full width. Chunks are a balanced split of N_core with sizes <= 512 (one
  PSUM bank per matmul).
- Expert weights stream through a 4-deep ring; their DMAs have no compute
  dependencies so they prefetch during the encoder phase. Decoder weights
  prefetch during the expert phase.
- Host: permute+transpose x, run 8 cores, inverse-permute the output.
"""

import ml_dtypes
import numpy as np

import concourse.bass as bass
import concourse.mybir as mybir
import concourse.tile as tile
from concourse import bacc
from concourse.bass_utils import run_bass_kernel_spmd

N_CORES = 8
N_CLS = 10
D_IN, D_H, D_BOT, D_EXP = 1024, 2048, 512, 1024

F32 = mybir.dt.float32
BF16 = mybir.dt.bfloat16
RELU = mybir.ActivationFunctionType.Relu
IDENT = mybir.ActivationFunctionType.Identity

CHUNK = 512  # max matmul moving-operand (free dim) size: one PSUM bank fp32


def _chunks(n, step=CHUNK):
    """Balanced split of n into ceil(n/step) near-equal pieces (all <= step)."""
    nch = -(-n // step)
    base, rem = divmod(n, nch)
    out = []
    s = 0
    for i in range(nch):
        sz = base + (1 if i < rem else 0)
        out.append((s, sz))
        s += sz
    return out


def _build(n_seg, n_core):
    """Build the SPMD program for per-class-per-core counts n_seg (sum=n_core)."""
    nc = bacc.Bacc()

    xt = nc.dram_tensor("xt", [D_IN, n_core], BF16, kind="ExternalInput")
    w1 = nc.dram_tensor("w1", [D_IN, D_H], BF16, kind="ExternalInput")
    b1 = nc.dram_tensor("b1", [128, D_H // 128], F32, kind="ExternalInput")
    w2 = nc.dram_tensor("w2", [D_H, D_BOT], BF16, kind="ExternalInput")
    b2 = nc.dram_tensor("b2", [128, D_BOT // 128], F32, kind="ExternalInput")
    ew1 = nc.dram_tensor("ew1", [N_CLS, D_BOT, D_EXP], BF16, kind="ExternalInput")
    eb1 = nc.dram_tensor("eb1", [128, N_CLS, D_EXP // 128], F32, kind="ExternalInput")
    ew2 = nc.dram_tensor("ew2", [N_CLS, D_EXP, D_BOT], BF16, kind="ExternalInput")
    eb2 = nc.dram_tensor("eb2", [128, N_CLS, D_BOT // 128], F32, kind="ExternalInput")
    dw1 = nc.dram_tensor("dw1", [D_BOT, D_H], BF16, kind="ExternalInput")
    db1 = nc.dram_tensor("db1", [128, D_H // 128], F32, kind="ExternalInput")
    dw2 = nc.dram_tensor("dw2", [D_H, D_IN], BF16, kind="ExternalInput")
    db2 = nc.dram_tensor("db2", [128, D_IN // 128], F32, kind="ExternalInput")
    out = nc.dram_tensor("out", [D_IN, n_core], F32, kind="ExternalOutput")

    segs = []  # (class e, col start, col len)
    s = 0
    for e in range(N_CLS):
        if n_seg[e] > 0:
            segs.append((e, s, n_seg[e]))
            s += n_seg[e]
    chunks = _chunks(n_core)
    XC_BUFS = (D_IN // 128) * min(len(chunks), 4)

    KT1, MT1 = D_IN // 128, D_H // 128     # enc1: 8, 16
    KT2, MT2 = D_H // 128, D_BOT // 128    # enc2: 16, 4
    KE1, ME1 = D_BOT // 128, D_EXP // 128  # exp1: 4, 8
    KE2, ME2 = D_EXP // 128, D_BOT // 128  # exp2: 8, 4
    KD1, MD1 = D_BOT // 128, D_H // 128    # dec1: 4, 16
    KD2, MD2 = D_H // 128, D_IN // 128     # dec2: 16, 8

    with tile.TileContext(nc) as tc:
        p_const = tc.alloc_tile_pool(name="const", bufs=1)
        p_ps = tc.alloc_tile_pool(name="ps", bufs=8, space="PSUM")

        def bias_tile(h, tag, shape):
            t = p_const.tile(shape, F32, tag=tag, name=tag)
            nc.sync.dma_start(out=t, in_=h[:])
            return t

        b1_t = bias_tile(b1, "b1", [128, MT1])
        b2_t = bias_tile(b2, "b2", [128, MT2])
        eb1_t = bias_tile(eb1, "eb1", [128, N_CLS, ME1])
        eb2_t = bias_tile(eb2, "eb2", [128, N_CLS, ME2])
        db1_t = bias_tile(db1, "db1", [128, MD1])
        db2_t = bias_tile(db2, "db2", [128, MD2])

        # bottleneck activations, SBUF-resident at full width
        p_e2 = tc.alloc_tile_pool(name="e2", bufs=1)
        p_h2 = tc.alloc_tile_pool(name="h2", bufs=1)
        e2_t = [p_e2.tile([128, n_core], BF16, tag=f"e2_{m}", name=f"e2_{m}")
                for m in range(D_BOT // 128)]
        h2_t = [p_h2.tile([128, n_core], BF16, tag=f"h2_{m}", name=f"h2_{m}")
                for m in range(D_BOT // 128)]

        # Expert pool is allocated BEFORE the encoder pool: its space never
        # overlaps encoder tiles, so expert-weight DMAs carry no false deps
        # and prefetch during the encoder phase.
        EW_BUFS = 4
        p_exp = tc.alloc_tile_pool(name="exp", bufs=1)
        e1_t = [p_exp.tile([128, n_core], BF16, tag=f"e1_{m}", name=f"e1_{m}")
                for m in range(D_EXP // 128)]
        # Preallocate the expert-weight ring HERE (before the encoder pool) so
        # the slots live below the encoder arena and the weight DMAs carry no
        # false WAR deps on encoder compute -> they prefetch from t~0.
        ew1_ring = [p_exp.tile([128, KE1, D_EXP], BF16, tag=f"ew1_{i}",
                               name=f"ew1_{i}") for i in range(EW_BUFS)]
        ew2_ring = [p_exp.tile([128, KE2, D_BOT], BF16, tag=f"ew2_{i}",
                               name=f"ew2_{i}") for i in range(EW_BUFS)]

        # ---------------- encoder (fused enc1+enc2 per chunk) -----------------
        p_enc = tc.alloc_tile_pool(name="enc", bufs=1)

        def load_xc(c0, cl):
            xc = []
            for k in range(KT1):
                t = p_enc.tile([128, CHUNK], BF16, tag="xc", name="xc", bufs=XC_BUFS)
                nc.sync.dma_start(out=t[:, :cl],
                                  in_=xt[k * 128:(k + 1) * 128, c0:c0 + cl])
                xc.append(t)
            return xc

        # x chunk 0 first so the PE can start quickly, then W1; W2 loads are
        # deferred past chunk-0 enc1 (not needed until enc2).
        xc = load_xc(*chunks[0])
        w1_t = []
        for k in range(KT1):
            t = p_enc.tile([128, D_H], BF16, tag=f"w1_{k}", name=f"w1_{k}")
            nc.sync.dma_start(out=t, in_=w1[k * 128:(k + 1) * 128, :])
            w1_t.append(t)
        w2_t = []

        for ci, (c0, cl) in enumerate(chunks):
            if ci > 0:
                xc = load_xc(c0, cl)
            h1c = []
            for m in range(MT1):
                ps = p_ps.tile([128, cl], F32, tag="ps", name="ps")
                for k in range(KT1):
                    nc.tensor.matmul(ps, w1_t[k][:, m * 128:(m + 1) * 128],
                                     xc[k][:, :cl],
                                     start=(k == 0), stop=(k == KT1 - 1))
                h = p_enc.tile([128, CHUNK], BF16, tag="h1c", name="h1c",
                               bufs=MT1)
                nc.scalar.activation(out=h[:, :cl], in_=ps, func=RELU,
                                     bias=b1_t[:, m:m + 1], scale=1.0)
                h1c.append(h)
            if ci == 0:
                for k in range(KT2):
                    t = p_enc.tile([128, D_BOT], BF16, tag=f"w2_{k}",
                                   name=f"w2_{k}")
                    nc.sync.dma_start(out=t, in_=w2[k * 128:(k + 1) * 128, :])
                    w2_t.append(t)
            for m in range(MT2):
                ps = p_ps.tile([128, cl], F32, tag="ps", name="ps")
                for k in range(KT2):
                    nc.tensor.matmul(ps, w2_t[k][:, m * 128:(m + 1) * 128],
                                     h1c[k][:, :cl],
                                     start=(k == 0), stop=(k == KT2 - 1))
                nc.scalar.activation(out=h2_t[m][:, c0:c0 + cl], in_=ps, func=RELU,
                                     bias=b2_t[:, m:m + 1], scale=1.0)

        p_enc.release()

        # Decoder pool reuses the released encoder space; its weight DMAs only
        # wait on the last encoder consumers, so they stream during the expert
        # phase.
        p_dec = tc.alloc_tile_pool(name="dec", bufs=1)
        # Triggered from the (otherwise idle) gpsimd sequencer: these waits on
        # the encoder's last consumers must not head-of-line-block the expert
        # weight stream on the sync sequencer.
        dw1_t = []
        for k in range(KD1):
            t = p_dec.tile([128, D_H], BF16, tag=f"dw1_{k}", name=f"dw1_{k}")
            nc.gpsimd.dma_start(out=t, in_=dw1[k * 128:(k + 1) * 128, :])
            dw1_t.append(t)
        dw2_t = []
        for k in range(KD2):
            t = p_dec.tile([128, D_IN], BF16, tag=f"dw2_{k}", name=f"dw2_{k}")
            nc.gpsimd.dma_start(out=t, in_=dw2[k * 128:(k + 1) * 128, :])
            dw2_t.append(t)

        # ---------------- experts: h2[512] -> e1[1024] -> e2[512] -------------
        for ei, (e, s0, sl) in enumerate(segs):
            ew1_t = ew1_ring[ei % EW_BUFS]
            nc.sync.dma_start(
                out=ew1_t, in_=ew1[e].rearrange("(a p) n -> p a n", p=128))
            ew2_t = ew2_ring[ei % EW_BUFS]
            nc.sync.dma_start(
                out=ew2_t, in_=ew2[e].rearrange("(a p) n -> p a n", p=128))

            for c0, cl in _chunks(sl):
                a, al = s0 + c0, cl
                for m in range(ME1):
                    ps = p_ps.tile([128, al], F32, tag="ps", name="ps")
                    for k in range(KE1):
                        nc.tensor.matmul(ps, ew1_t[:, k, m * 128:(m + 1) * 128],
                                         h2_t[k][:, a:a + al],
                                         start=(k == 0), stop=(k == KE1 - 1))
                    nc.scalar.activation(out=e1_t[m][:, a:a + al], in_=ps,
                                         func=RELU, bias=eb1_t[:, e, m:m + 1],
                                         scale=1.0)
                for m in range(ME2):
                    ps = p_ps.tile([128, al], F32, tag="ps", name="ps")
                    for k in range(KE2):
                        nc.tensor.matmul(ps, ew2_t[:, k, m * 128:(m + 1) * 128],
                                         e1_t[k][:, a:a + al],
                                         start=(k == 0), stop=(k == KE2 - 1))
                    nc.scalar.activation(out=e2_t[m][:, a:a + al], in_=ps,
                                         func=RELU, bias=eb2_t[:, e, m:m + 1],
                                         scale=1.0)

        # ---------------- decoder (fused dec1+dec2 per chunk) -----------------
        for c0, cl in chunks:
            d1c = []
            for m in range(MD1):
                ps = p_ps.tile([128, cl], F32, tag="ps", name="ps")
                for k in range(KD1):
                    nc.tensor.matmul(ps, dw1_t[k][:, m * 128:(m + 1) * 128],
                                     e2_t[k][:, c0:c0 + cl],
                                     start=(k == 0), stop=(k == KD1 - 1))
                d = p_dec.tile([128, CHUNK], BF16, tag="d1c", name="d1c",
                               bufs=MD1)
                nc.scalar.activation(out=d[:, :cl], in_=ps, func=RELU,
                                     bias=db1_t[:, m:m + 1], scale=1.0)
                d1c.append(d)
            for m in range(MD2):
                ps = p_ps.tile([128, cl], F32, tag="ps", name="ps")
                for k in range(KD2):
                    nc.tensor.matmul(ps, dw2_t[k][:, m * 128:(m + 1) * 128],
                                     d1c[k][:, :cl],
                                     start=(k == 0), stop=(k == KD2 - 1))
                o_t = p_dec.tile([128, CHUNK], F32, tag="o", name="o", bufs=4)
                nc.scalar.activation(out=o_t[:, :cl], in_=ps, func=IDENT,
                                     bias=db2_t[:, m:m + 1], scale=1.0)
                nc.sync.dma_start(out=out[m * 128:(m + 1) * 128, c0:c0 + cl],
                                  in_=o_t[:, :cl])

        p_dec.release()
        p_exp.release()
        p_h2.release()
        p_e2.release()
        p_ps.release()
        p_const.release()

    nc.finalize()
    return nc


_CACHE = {}


def _get_nc(n_seg, n_core):
    key = tuple(n_seg)
    if key not in _CACHE:
        _CACHE[key] = _build(n_seg, n_core)
    return _CACHE[key]


def _bf16(a):
    return np.ascontiguousarray(np.asarray(a, np.float32).astype(ml_dtypes.bfloat16))


def _f32(a):
    return np.ascontiguousarray(np.asarray(a, np.float32))


def _bias_fm(b, mt):
    """[mt*128] -> [128, mt] feature-major (partition-contiguous) layout."""
    return np.ascontiguousarray(np.asarray(b, np.float32).reshape(mt, 128).T)


def _ebias_fm(b, mt):
    """[N_CLS, mt*128] -> [128, N_CLS, mt]."""
    a = np.asarray(b, np.float32).reshape(N_CLS, mt, 128)
    return np.ascontiguousarray(a.transpose(2, 0, 1))


def kernel(x, labels, W1, b1, W2, b2, EW1, Eb1, EW2, Eb2, DW1, Db1, DW2, Db2):
    x = np.asarray(x, dtype=np.float32)
    labels_np = np.asarray(labels).astype(np.int64)
    B = x.shape[0]

    counts = np.bincount(labels_np, minlength=N_CLS)
    n_seg = [int(-(-int(c) // N_CORES)) for c in counts]  # ceil(c/8)
    n_core = int(sum(n_seg))

    # assign tokens: class e sorted tokens padded to 8*n_seg[e], row j -> core j
    order = np.argsort(labels_np, kind="stable")
    idx_by_class = np.split(order, np.cumsum(counts)[:-1])
    core_tok = np.full((N_CORES, n_core), -1, dtype=np.int64)
    off = 0
    for e in range(N_CLS):
        ne = n_seg[e]
        if ne == 0:
            continue
        padded = np.full(N_CORES * ne, -1, dtype=np.int64)
        padded[:counts[e]] = idx_by_class[e]
        core_tok[:, off:off + ne] = padded.reshape(N_CORES, ne)
        off += ne

    weights = {
        "w1": _bf16(W1), "b1": _bias_fm(b1, D_H // 128),
        "w2": _bf16(W2), "b2": _bias_fm(b2, D_BOT // 128),
        "ew1": _bf16(EW1), "eb1": _ebias_fm(Eb1, D_EXP // 128),
        "ew2": _bf16(EW2), "eb2": _ebias_fm(Eb2, D_BOT // 128),
        "dw1": _bf16(DW1), "db1": _bias_fm(Db1, D_H // 128),
        "dw2": _bf16(DW2), "db2": _bias_fm(Db2, D_IN // 128),
    }

    x_bf = x.astype(ml_dtypes.bfloat16)
    in_maps = []
    for j in range(N_CORES):
        ids = core_tok[j]
        valid = ids >= 0
        xc = np.zeros((n_core, D_IN), dtype=ml_dtypes.bfloat16)
        xc[valid] = x_bf[ids[valid]]
        im = {"xt": np.ascontiguousarray(xc.T)}
        im.update(weights)
        in_maps.append(im)

    nc = _get_nc(n_seg, n_core)
    res = run_bass_kernel_spmd(nc, in_maps, core_ids=list(range(N_CORES)))

    out = np.empty((B, D_IN), dtype=np.float32)
    for j in range(N_CORES):
        oc = res.results[j]["out"]  # [D_IN, n_core]
        ids = core_tok[j]
        valid = ids >= 0
        out[ids[valid]] = oc.T[valid]
    return out
